# revision 1
# baseline (speedup 1.0000x reference)
"""Block-diagonal 4-layer MLP (8 experts) on 8 Trainium2 NeuronCores.

Expert-parallel: core e computes expert e's chain
    h = relu(W0_e @ x.T + b0_e); h = relu(W1_e @ h + b1_e);
    h = relu(W2_e @ h + b2_e);   y_e.T = W3_e @ h + b3_e
with activations stored transposed [features, batch] so the tensor engine
streams batch as the moving free dim. Weights are fed pre-transposed
(W_e.T = [in, out]) so lhsT tiles slice directly. fp32r matmuls (full PE
rate), bias+ReLU fused into one ScalarE/VectorE op reading PSUM.

Startup: bf16 dummy matmuls on a zeroed SBUF tile keep the PE array busy
(HAM warm-up) while input DMAs stream in, and a dummy ReLU preloads the
ACT function table. Input DMAs are chunked and ordered to match the
order the PE consumes them (x first half, W0 halves, x second half, then
W1..W3 interleaved with biases); a few more dummy matmuls between early
L0 groups absorb the remaining DMA-feed gaps. Layers alternate batch
halves (n0/n1) so each layer boundary's activation latency is hidden by
the other half's matmuls. Cost-model (TimelineSim) per-core time:
~30.5 us vs a ~20.4 us pure-matmul floor; the rest is serialized input
DMA feed and the fixed store/semaphore tail.
"""

import sys

import numpy as np

for _p in ("/opt/trn_rl_repo", "/root/.axon_site/_ro/trn_rl_repo"):
    if _p not in sys.path:
        sys.path.append(_p)

import concourse.bass as bass  # noqa: E402
import concourse.tile as tile  # noqa: E402
from concourse import bacc, mybir  # noqa: E402
from concourse.bass_utils import run_bass_kernel_spmd  # noqa: E402

N_PAR = 8
IN, HID, OUT, B = 256, 512, 256, 1024
P = 128
BN = 512  # batch chunk = max fp32 moving free dim = one PSUM bank
NB = B // BN
F32 = mybir.dt.float32
F32R = mybir.dt.float32r
# (K, M) of each layer's W^T
DIMS = [(IN, HID), (HID, HID), (HID, HID), (HID, OUT)]
WARMUP_MMS = 6

_cached_nc = None
LAST_RESULTS = None


DEFAULT_DMA_PLAN = [
    "x0", "w00", "w01", "x1", "b0",
    "w10", "b1", "w11", "w20", "b2", "w21", "w30", "b3", "w31",
]
# (layer, batch chunk, act-engine parity)
DEFAULT_SCHED_PLAN = [
    (0, 0, 0), (0, 1, 0), (1, 0, 1), (1, 1, 0),
    (2, 0, 1), (2, 1, 0), (3, 0, 0), (3, 1, 0),
]


def _build(warmup_mms=WARMUP_MMS, fill1=1, fill2=2, l3_split=True,
           dma_plan=None, sched_plan=None):
    nc = bacc.Bacc(
        trn_type="TRN2",
        target_bir_lowering=False,
        debug=False,
        num_devices=N_PAR,
    )
    xt = nc.dram_tensor("xt", [IN, B], F32R, kind="ExternalInput").ap()
    w_aps = [
        nc.dram_tensor(f"w{l}t", [k, m], F32R, kind="ExternalInput").ap()
        for l, (k, m) in enumerate(DIMS)
    ]
    b_aps = [
        nc.dram_tensor(f"b{l}", [m], F32, kind="ExternalInput").ap()
        for l, (_, m) in enumerate(DIMS)
    ]
    yt = nc.dram_tensor("yt", [OUT, B], F32, kind="ExternalOutput").ap()
    yt_t = yt.rearrange("(mt p) b -> p mt b", p=P)

    with tile.TileContext(nc) as tc:
        with (
            tc.tile_pool(name="w", bufs=1) as wpool,
            tc.tile_pool(name="acts", bufs=1) as apool,
            tc.tile_pool(name="outs", bufs=4) as opool,
            tc.tile_pool(name="psum", bufs=7, space="PSUM") as psum,
            tc.tile_pool(name="warm", bufs=1, space="PSUM") as warmpool,
        ):
            # --- PE warmup + ACT table preload (no DMA dependency) ---
            # bf16 dummy matmuls: same PE streaming rate as fp32r, no
            # fp32r-producer rounding constraint on the memset
            warm_src = apool.tile([P, BN], mybir.dt.bfloat16, tag="warmsrc")
            nc.vector.memset(warm_src[:], 0.0)
            warm_ps = warmpool.tile([P, BN], F32, tag="warmps")
            for _ in range(warmup_mms):
                nc.tensor.matmul(
                    warm_ps[:], warm_src[:, :P], warm_src[:], start=True, stop=True
                )
            warm_act = apool.tile([P, 1], F32, tag="warmact")
            nc.scalar.activation(
                warm_act[:], warm_src[:, :1],
                mybir.ActivationFunctionType.Relu,
            )

            # --- SBUF allocations ---
            x_sb = apool.tile([P, IN // P, B], F32R, tag="x")
            w_sb = [
                wpool.tile([P, k // P, m], F32R, tag=f"w{l}", name=f"w{l}")
                for l, (k, m) in enumerate(DIMS)
            ]
            b_sb = [
                wpool.tile([P, m // P], F32, tag=f"b{l}", name=f"b{l}")
                for l, (_, m) in enumerate(DIMS)
            ]
            h_sb = [
                apool.tile([P, m // P, B], F32R, tag=f"h{l}", name=f"h{l}")
                for l, (_, m) in enumerate(DIMS[:-1])
            ]

            # --- input DMAs, chunked + ordered by first use ---
            xt_t = xt.rearrange("(kt p) b -> p kt b", p=P)
            w_t = [
                w_aps[l].rearrange("(kt p) m -> p kt m", p=P) for l in range(len(DIMS))
            ]
            b_t = [
                b_aps[l].rearrange("(mt p) -> p mt", p=P) for l in range(len(DIMS))
            ]

            # alternate the two independent HWDGE queues (SP / ACT): on real
            # hardware the transfers run concurrently, halving the input feed
            # latency (the cost model serializes them on one device either way)
            _dma_eng = [nc.sync, nc.scalar]

            def dma_x(n, q):
                sl = slice(n * BN, (n + 1) * BN)
                _dma_eng[q].dma_start(x_sb[:, :, sl], xt_t[:, :, sl])

            def dma_w(l, half, q):
                m = DIMS[l][1]
                sl = slice(half * (m // 2), (half + 1) * (m // 2))
                _dma_eng[q].dma_start(w_sb[l][:, :, sl], w_t[l][:, :, sl])

            def dma_b(l, q):
                _dma_eng[q].dma_start(b_sb[l][:], b_t[l])

            # only the startup DMAs (ACT engine idle there); later DMAs stay
            # on SP so descriptor generation never delays activations
            act_q_idx = {1, 3}
            for i, tok in enumerate(dma_plan or DEFAULT_DMA_PLAN):
                kind, a, q = tok[0], int(tok[1]), 1 if i in act_q_idx else 0
                if kind == "x":
                    dma_x(a, q)
                elif kind == "b":
                    dma_b(a, q)
                else:
                    dma_w(a, int(tok[2]), q)

            def relu_store(idx, dst, ps, bias, func):
                # bias(+relu) from PSUM into SBUF, alternating engines
                if idx % 2 == 0:
                    nc.scalar.activation(dst, ps, func, bias=bias)
                else:
                    if func == mybir.ActivationFunctionType.Relu:
                        nc.vector.tensor_scalar(
                            dst, ps, bias, 0.0,
                            mybir.AluOpType.add, mybir.AluOpType.max,
                        )
                    else:
                        nc.vector.tensor_scalar(
                            dst, ps, bias, None, mybir.AluOpType.add
                        )

            relu = mybir.ActivationFunctionType.Relu
            ident = mybir.ActivationFunctionType.Identity

            def dummy_mms(count):
                # PE filler while DMAs stream in: keeps the array warm,
                # no data dependencies
                for _ in range(count):
                    nc.tensor.matmul(
                        warm_ps[:], warm_src[:, :P], warm_src[:],
                        start=True, stop=True,
                    )

            def layer_chunk(l, n, idx0, ms=None):
                # m-groups of layer l on batch chunk n
                src = x_sb if l == 0 else h_sb[l - 1]
                last = l == len(DIMS) - 1
                kt, mt = DIMS[l][0] // P, DIMS[l][1] // P
                bsl = slice(n * BN, (n + 1) * BN)
                for m in ms if ms is not None else range(mt):
                    bias = b_sb[l][:, m : m + 1]
                    if last and n == NB - 1 and l3_split:
                        # final group via two 256-wide PSUM banks: the two
                        # bias-adds run on ScalarE and VectorE in parallel
                        # (different banks), then one store for the full tile
                        hw_ = BN // 2
                        o = opool.tile([P, BN], F32, tag="o", name="o")
                        for h2 in range(2):
                            lo = n * BN + h2 * hw_
                            ps = psum.tile([P, hw_], F32, tag="ps", name="ps")
                            for k in range(kt):
                                nc.tensor.matmul(
                                    ps[:],
                                    w_sb[l][:, k, m * P : (m + 1) * P],
                                    src[:, k, lo : lo + hw_],
                                    start=(k == 0),
                                    stop=(k == kt - 1),
                                )
                            osl = o[:, h2 * hw_ : (h2 + 1) * hw_]
                            if h2 == 0:
                                nc.scalar.activation(osl, ps[:], ident, bias=bias)
                            else:
                                nc.vector.tensor_scalar(
                                    osl, ps[:], bias, None, mybir.AluOpType.add
                                )
                        nc.sync.dma_start(yt_t[:, m, bsl], o[:])
                        continue
                    ps = psum.tile([P, BN], F32, tag="ps", name="ps")
                    for k in range(kt):
                        nc.tensor.matmul(
                            ps[:],
                            w_sb[l][:, k, m * P : (m + 1) * P],
                            src[:, k, bsl],
                            start=(k == 0),
                            stop=(k == kt - 1),
                        )
                    if last:
                        o = opool.tile([P, BN], F32, tag="o", name="o")
                        relu_store(idx0 + m, o[:], ps[:], bias, ident)
                        nc.sync.dma_start(yt_t[:, m, bsl], o[:])
                    else:
                        relu_store(idx0 + m, h_sb[l][:, m, bsl], ps[:], bias, relu)
                    if l == 0 and n == 0 and m == 1:
                        dummy_mms(fill1)
                if l == 0 and n == 0 and (ms is None or ms[-1] == mt - 1):
                    dummy_mms(fill2)

            for l, n, idx0 in (sched_plan or DEFAULT_SCHED_PLAN):
                layer_chunk(l, n, idx0)
    nc.compile()
    return nc


def kernel(_trace=False, **inputs):
    global _cached_nc, LAST_RESULTS
    x = np.ascontiguousarray(inputs["x"], dtype=np.float32)
    if _cached_nc is None:
        _cached_nc = _build()
    nc = _cached_nc

    xt = np.ascontiguousarray(x.T)
    out_sizes = [HID, HID, HID, OUT]
    in_sizes = [IN, HID, HID, HID]
    in_maps = []
    for e in range(N_PAR):
        m = {"xt": xt}
        for l in range(4):
            r0, c0 = e * out_sizes[l], e * in_sizes[l]
            blk = inputs[f"W{l}"][r0 : r0 + out_sizes[l], c0 : c0 + in_sizes[l]]
            m[f"w{l}t"] = np.ascontiguousarray(np.asarray(blk).T, dtype=np.float32)
            m[f"b{l}"] = np.ascontiguousarray(
                np.asarray(inputs[f"b{l}"][r0 : r0 + out_sizes[l]]), dtype=np.float32
            )
        in_maps.append(m)

    try:
        res = run_bass_kernel_spmd(
            nc, in_maps, core_ids=list(range(N_PAR)), trace=_trace
        )
    except Exception:
        # transient device errors (e.g. NRT_EXEC_UNIT_UNRECOVERABLE) clear
        # after the runtime re-initializes; retry once
        import time

        time.sleep(30)
        res = run_bass_kernel_spmd(
            nc, in_maps, core_ids=list(range(N_PAR)), trace=_trace
        )
    LAST_RESULTS = res
    y_p = np.concatenate(
        [res.results[e]["yt"].T for e in range(N_PAR)], axis=1
    ).astype(np.float32)
    x_p = np.tile(x, (1, N_PAR)).astype(np.float32)
    return (y_p, x_p)



# revision 4
# speedup vs baseline: 1.0373x; 1.0373x over previous
"""Block-diagonal 4-layer MLP (8 experts) on 8 Trainium2 NeuronCores.

Expert-parallel. Layers 0-1 run fp8e4m3 DoubleRow matmuls (2 k-planes per
instruction, 0.5 cycles/row — 4x fp32r throughput in the TRN2 cost model);
layers 2-3 run bf16. The fp8 layers use value+residual quantization so each
product is computed to ~2^-8 accuracy as three PSUM-accumulated terms:

    W·σ ≈ A + B   (A = fp8(W·σ), B = fp8(W·σ − A))
    h·τ ≈ H + E   (H = fp8(ht·s), E = ht·s − H)
    psum = A@H + B@H + A@E   (all at scale σ·τ)

x's pair is built on the host (free). h1 needs three ops (all 2-stage, so
each is placeable on ScalarE or DVE; Pool's tensor ops don't pass the BIR
verifier): op1 ht = relu(psum + b·στ) (bf16, PSUM scale riding), op2
H = fp8(ht·s), op3 E = ht·s − H (scalar_tensor_tensor). h2/h3 are single
relu-adds kept bf16 at the riding scale σ₁τ₁ = 2^18 — bf16's exponent range
makes the scale free — and layer 3's bias-add unscales by 2^-18 and stores
y in bf16, upcast on the host.

The matmul stream is scheduled as blocks (fp8 term x batch-chunk, bf16
chunk), so a late input only delays its own term; A/B and xh/xe are packed
into shared DRAM tensors to minimize serialized HWDGE descriptor-generation
slots at startup. A tiny memset feeds dummy matmuls that ramp the PE
p-state under the first DMAs; the last two output stores issue from
different queues so their SEQ-side costs overlap.
"""

import sys

import numpy as np

for _p in ("/opt/trn_rl_repo", "/root/.axon_site/_ro/trn_rl_repo"):
    if _p not in sys.path:
        sys.path.append(_p)

import ml_dtypes  # noqa: E402

import concourse.bass as bass  # noqa: E402, F401
import concourse.tile as tile  # noqa: E402
from concourse import bacc, mybir  # noqa: E402
from concourse.bass_utils import run_bass_kernel_spmd  # noqa: E402

N_PAR = 8
IN, HID, OUT, B = 256, 512, 256, 1024
P = 128
BN = 512  # batch chunk = one fp32 PSUM bank
NB = B // BN
F32 = mybir.dt.float32
BF16 = mybir.dt.bfloat16
BF16_NP = ml_dtypes.bfloat16
FP8 = mybir.dt.float8e4
FP8_NP = ml_dtypes.float8_e4m3
DR = mybir.MatmulPerfMode.DoubleRow
ADD = mybir.AluOpType.add
MAX = mybir.AluOpType.max
MULT = mybir.AluOpType.mult
SUB = mybir.AluOpType.subtract
# (K, M) of each layer's W^T
DIMS = [(IN, HID), (HID, HID), (HID, HID), (HID, OUT)]
BIAS_OFF = [0, 4, 8, 12]  # column offsets in the merged bias tile [P, 14]

# power-of-2 scales; ranges sized for W ~ U(+-1/sqrt(in_f)), x ~ N(0,1)
SIGMA = [2.0**12, 2.0**12]  # fp8 weight scale, layers 0-1
TAU = [2.0**4, 2.0**6]  # fp8 act scale: x, h1
S0 = TAU[1] / (SIGMA[0] * TAU[0])  # h1 rescale in op2/op3
RIDE = SIGMA[1] * TAU[1]  # h2/h3/psum3 ride at this scale (2^18)

_cached_nc = None
LAST_RESULTS = None

# DMA plan: (kind, lo, hi, queue). kind xq: packed [xh | xe] col range;
# ar{l}: packed [A | R] col range; w2/w3/bm whole. queue 0=SP, 1=ACT.
DEFAULT_DMA_PLAN = [
    ("ar0", 0, 512, 1), ("xa", 0, 0, 0), ("ar0", 512, 1024, 1),
    ("xb", 0, 1536, 0),
    ("ar1", 0, 512, 0), ("bm", 0, 0, 0), ("ar1", 512, 1024, 0),
    ("w2", 0, 0, 0), ("w3", 0, 0, 0),
]

# blocks: ("mm", l, term, n) fp8 term-block (l 0-1); ("acts1", n) h1 ops;
# ("acts2", n) h2 op; ("mmb", n) bf16 L2 chunk; ("acts3", n) h3 op;
# ("l3", n); ("fill", count)
DEFAULT_BLOCKS = [
    ("mm", 0, 0, 0), ("fill", 1), ("mm", 0, 1, 0), ("fill", 1),
    ("mm", 0, 2, 0), ("acts1a", 0),
    ("mm", 0, 0, 1), ("mm", 0, 1, 1), ("mm", 0, 2, 1), ("acts1a", 1),
    ("acts1b", 0), ("mm", 1, 0, 0), ("mm", 1, 1, 0), ("acts1b", 1),
    ("mm", 1, 2, 0), ("acts2", 0),
    ("mm", 1, 0, 1), ("mm", 1, 1, 1), ("mm", 1, 2, 1), ("acts2", 1),
    ("mmb", 0), ("acts3", 0),
    ("mmb", 1), ("l3", 0), ("acts3", 1),
    ("l3", 1, 0, 256, False), ("l3", 1, 256, 512, True),
]

# engine assignment (a=ScalarE, v=DVE only — Pool can't run these ops).
# op2 'A' = direct H from PSUM on ScalarE: relu(psum·s + b·τ) in one op
# (relu commutes with the positive scale), skipping the ht dependency.
OP1 = "avav avav".replace(" ", "")  # h1 op1 (ht), tile = 4*n + m
OP2 = "vaAa aAva".replace(" ", "")  # h1 op2 (H)
H2E = "avaa avav".replace(" ", "")  # h2 tiles
H3E = "avav avav".replace(" ", "")  # h3 tiles
FINE = "aavv"  # 4 L3 bias-adds
WARM = (10, 64)  # (dummy matmul count, narrow width for the first 5)


def _build(warm=WARM, dma_plan=None, blocks=None, op1=OP1, op2=OP2,
           h2e=H2E, h3e=H3E, fine=FINE, psum_bufs=7, kouter=False):
    nc = bacc.Bacc(
        trn_type="TRN2",
        target_bir_lowering=False,
        debug=False,
        num_devices=N_PAR,
    )
    ar_aps = [
        nc.dram_tensor(f"ar{l}", [k, 2 * m], FP8, kind="ExternalInput").ap()
        for l, (k, m) in enumerate(DIMS[:2])
    ]
    w2_ap = nc.dram_tensor("w2", [DIMS[2][0], DIMS[2][1]], BF16,
                           kind="ExternalInput").ap()
    w3_ap = nc.dram_tensor("w3", [DIMS[3][0], DIMS[3][1]], BF16,
                           kind="ExternalInput").ap()
    xa_ap = nc.dram_tensor("xa", [IN, BN], FP8, kind="ExternalInput").ap()
    xb_ap = nc.dram_tensor("xb", [IN, 2 * B - BN], FP8,
                           kind="ExternalInput").ap()
    bm_ap = nc.dram_tensor("bm", [P, 18], F32, kind="ExternalInput").ap()
    yt = nc.dram_tensor("yt", [OUT, B], BF16, kind="ExternalOutput").ap()
    yt_t = yt.rearrange("(mt p) b -> p mt b", p=P)

    with tile.TileContext(nc) as tc:
        with (
            tc.tile_pool(name="w", bufs=1) as wpool,
            tc.tile_pool(name="acts", bufs=1) as apool,
            tc.tile_pool(name="outs", bufs=4) as opool,
            tc.tile_pool(name="psum", bufs=psum_bufs, space="PSUM") as psum,
            tc.tile_pool(name="warm", bufs=1, space="PSUM") as warmpool,
        ):
            # --- PE warmup (p-state ramp) + ACT table preload ---
            wn, ww = warm
            ww = max(ww, P)
            warm_src = apool.tile([P, BN], BF16, tag="warmsrc")
            nc.vector.memset(warm_src[:, :ww], 0.0)
            if ww < BN:
                nc.vector.memset(warm_src[:, ww:], 0.0)
            warm_ps = warmpool.tile([P, BN], F32, tag="warmps")

            def dummy_mms(count, width=None):
                for _ in range(count):
                    nc.tensor.matmul(
                        warm_ps[:, : (width or ww)], warm_src[:, :P],
                        warm_src[:, : (width or ww)],
                        start=True, stop=True,
                    )

            # narrow dummies start the PE busy-clock ASAP and bridge the
            # second memset's latency; the rest run full-width to accumulate
            # ramp time under the input DMAs
            nn = min(wn, 5)
            dummy_mms(nn)
            if wn > nn:
                dummy_mms(wn - nn, BN)
            warm_act = apool.tile([P, 1], F32, tag="warmact")
            nc.scalar.activation(
                warm_act[:], warm_src[:, :1],
                mybir.ActivationFunctionType.Relu,
            )

            # --- SBUF allocations ---
            ar_sb = [
                wpool.tile([P, k // P, 2 * m], FP8, tag=f"ar{l}", name=f"ar{l}")
                for l, (k, m) in enumerate(DIMS[:2])
            ]
            w2_sb = wpool.tile([P, DIMS[2][0] // P, DIMS[2][1]], BF16, tag="w2")
            w3_sb = wpool.tile([P, DIMS[3][0] // P, DIMS[3][1]], BF16, tag="w3")
            xa_sb = apool.tile([P, IN // P, BN], FP8, tag="xa")
            xb_sb = apool.tile([P, IN // P, 2 * B - BN], FP8, tag="xb")
            bm_sb = wpool.tile([P, 18], F32, tag="bm")
            ht_sb = apool.tile([P, HID // P, B], BF16, tag="ht")
            hh_sb = apool.tile([P, HID // P, B], FP8, tag="hh")
            he_sb = apool.tile([P, HID // P, B], FP8, tag="he")
            h2_sb = apool.tile([P, HID // P, B], BF16, tag="h2")
            h3_sb = apool.tile([P, HID // P, B], BF16, tag="h3")

            ar_t = [ar_aps[l].rearrange("(kt p) m -> p kt m", p=P)
                    for l in range(2)]
            w2_t = w2_ap.rearrange("(kt p) m -> p kt m", p=P)
            w3_t = w3_ap.rearrange("(kt p) m -> p kt m", p=P)
            xa_t = xa_ap.rearrange("(kt p) b -> p kt b", p=P)
            xb_t = xb_ap.rearrange("(kt p) b -> p kt b", p=P)
            _q = [nc.sync, nc.scalar]

            for kind, lo, hi, q in dma_plan or DEFAULT_DMA_PLAN:
                eng = _q[q]
                if kind == "bm":
                    eng.dma_start(bm_sb[:], bm_ap)
                elif kind == "w2":
                    eng.dma_start(w2_sb[:], w2_t)
                elif kind == "w3":
                    eng.dma_start(w3_sb[:], w3_t)
                elif kind == "xa":
                    eng.dma_start(xa_sb[:], xa_t)
                elif kind == "xb":
                    eng.dma_start(xb_sb[:, :, lo:hi], xb_t[:, :, lo:hi])
                else:
                    l = int(kind[2])
                    eng.dma_start(ar_sb[l][:, :, lo:hi], ar_t[l][:, :, lo:hi])

            relu = mybir.ActivationFunctionType.Relu
            engs = {"v": nc.vector}
            pstate = {}

            def mm_block(l, term, n):
                """One fp8 term-block: 4 m-groups x kp DoubleRow matmuls."""
                kt = DIMS[l][0] // P
                kp = kt // 2
                m_w = DIMS[l][1]
                bsl = slice(n * BN, (n + 1) * BN)
                nmm = 3 * kp
                for m in range(4):
                    key = (l, n, m)
                    if key not in pstate:
                        pstate[key] = [
                            psum.tile([P, BN], F32, tag="ps", name="ps"), 0
                        ]
                    ps, cnt = pstate[key]
                    for k in range(kp):
                        if term == 0:  # A (x) H
                            wsl = slice(m * P, (m + 1) * P)
                        elif term == 1:  # B (x) H
                            wsl = slice(m_w + m * P, m_w + (m + 1) * P)
                        else:  # A (x) E
                            wsl = slice(m * P, (m + 1) * P)
                        if l == 0:
                            # xa = xh n0; xb = [xh n1 | xe n0 | xe n1]
                            if term != 2 and n == 0:
                                xs = xa_sb[:, 2 * k : 2 * k + 2, :]
                            else:
                                xoff = 0 if term != 2 else BN * (1 + n)
                                xs = xb_sb[:, 2 * k : 2 * k + 2,
                                           xoff : xoff + BN]
                        else:
                            xsb = he_sb if term == 2 else hh_sb
                            xs = xsb[:, 2 * k : 2 * k + 2, bsl]
                        nc.tensor.matmul(
                            ps[:],
                            ar_sb[l][:, 2 * k : 2 * k + 2, wsl],
                            xs,
                            start=(cnt == 0),
                            stop=(cnt == nmm - 1),
                            perf_mode=DR,
                        )
                        cnt += 1
                    pstate[key][1] = cnt

            def op1_issue(eng, dst, ps, bias):
                if eng == "a":
                    nc.scalar.activation(dst, ps[:], relu, bias=bias)
                else:
                    engs[eng].tensor_scalar(dst, ps[:], bias, 0.0, ADD, MAX)

            def acts1a_block(n):
                # H wave first (the critical dependency), then ht wave.
                # op2 'A' reads PSUM directly: H = relu(psum*s + b*tau).
                bsl = slice(n * BN, (n + 1) * BN)
                s = float(S0)
                for m in range(4):
                    t = 4 * n + m
                    ps = pstate[(0, n, m)][0]
                    hh_o = hh_sb[:, m, bsl]
                    if op2[t] == "A":
                        nc.scalar.activation(
                            hh_o, ps[:], relu,
                            bias=bm_sb[:, 14 + m : 15 + m], scale=s,
                        )
                for m in range(4):
                    t = 4 * n + m
                    ps = pstate[(0, n, m)][0]
                    bias = bm_sb[:, m : m + 1]
                    op1_issue(op1[t], ht_sb[:, m, bsl], ps, bias)
                for m in range(4):
                    t = 4 * n + m
                    pstate.pop((0, n, m))
                    ht = ht_sb[:, m, bsl]
                    hh_o = hh_sb[:, m, bsl]
                    if op2[t] == "A":
                        pass
                    elif op2[t] == "a":
                        nc.scalar.activation(
                            hh_o, ht, mybir.ActivationFunctionType.Copy,
                            scale=s,
                        )
                    else:
                        engs[op2[t]].tensor_scalar(hh_o, ht, s, None, MULT)

            def acts1b_block(n):
                # op3 wave (E — only needed by the A@E term, issued late)
                bsl = slice(n * BN, (n + 1) * BN)
                s = float(S0)
                for m in range(4):
                    nc.vector.scalar_tensor_tensor(
                        he_sb[:, m, bsl], ht_sb[:, m, bsl], s,
                        hh_sb[:, m, bsl], MULT, SUB,
                    )

            def acts2_block(n):
                bsl = slice(n * BN, (n + 1) * BN)
                for m in range(4):
                    ps = pstate.pop((1, n, m))[0]
                    bias = bm_sb[:, 4 + m : 4 + m + 1]
                    op1_issue(h2e[4 * n + m], h2_sb[:, m, bsl], ps, bias)

            def mmb_block(n):
                """bf16 L2 chunk: 4 m-groups x 4 k matmuls."""
                kt = DIMS[2][0] // P
                bsl = slice(n * BN, (n + 1) * BN)
                for m in range(4):
                    pstate[(2, n, m)] = [
                        psum.tile([P, BN], F32, tag="ps", name="ps"), 0
                    ]
                ordr = ([(k, m) for k in range(kt) for m in range(4)]
                        if kouter else
                        [(k, m) for m in range(4) for k in range(kt)])
                for k, m in ordr:
                    msl = slice(m * P, (m + 1) * P)
                    nc.tensor.matmul(
                        pstate[(2, n, m)][0][:],
                        w2_sb[:, k, msl], h2_sb[:, k, bsl],
                        start=(k == 0), stop=(k == kt - 1),
                    )

            def acts3_block(n):
                bsl = slice(n * BN, (n + 1) * BN)
                for m in range(4):
                    ps = pstate.pop((2, n, m))[0]
                    bias = bm_sb[:, 8 + m : 8 + m + 1]
                    op1_issue(h3e[4 * n + m], h3_sb[:, m, bsl], ps, bias)

            def l3_chunk(n, lo=0, hi=BN, last=False):
                """bf16 layer 3 on cols [n*BN+lo, n*BN+hi), store pipelined.
                The last piece's two stores go out on different queues so
                their SEQ-side issue overlaps."""
                kt = DIMS[3][0] // P
                w_ = hi - lo
                bsl = slice(n * BN + lo, n * BN + hi)
                pss = [psum.tile([P, w_], F32, tag="ps", name="ps")
                       for _ in range(2)]
                ordr = ([(k, m) for k in range(kt) for m in range(2)]
                        if kouter else
                        [(k, m) for m in range(2) for k in range(kt)])
                for k, m in ordr:
                    msl = slice(m * P, (m + 1) * P)
                    nc.tensor.matmul(
                        pss[m][:], w3_sb[:, k, msl], h3_sb[:, k, bsl],
                        start=(k == 0), stop=(k == kt - 1),
                    )
                o = opool.tile([P, 2, w_], BF16, tag="o", name="o")
                for m in range(2):
                    bias = bm_sb[:, 12 + m : 12 + m + 1]
                    ps = pss[m]
                    e = fine[n * 2 + m]
                    if e == "a":
                        nc.scalar.activation(
                            o[:, m], ps[:],
                            mybir.ActivationFunctionType.Identity,
                            bias=bias, scale=float(1.0 / RIDE),
                        )
                    else:
                        engs[e].tensor_scalar(
                            o[:, m], ps[:], float(1.0 / RIDE), bias, MULT, ADD
                        )
                # one store covers both m-groups (their fins ran in parallel)
                nc.sync.dma_start(yt_t[:, :, bsl], o[:])

            for blk in blocks or DEFAULT_BLOCKS:
                if blk[0] == "mm":
                    mm_block(blk[1], blk[2], blk[3])
                elif blk[0] == "acts1a":
                    acts1a_block(blk[1])
                elif blk[0] == "acts1b":
                    acts1b_block(blk[1])
                elif blk[0] == "acts2":
                    acts2_block(blk[1])
                elif blk[0] == "mmb":
                    mmb_block(blk[1])
                elif blk[0] == "acts3":
                    acts3_block(blk[1])
                elif blk[0] == "l3":
                    if len(blk) > 2:
                        l3_chunk(blk[1], blk[2], blk[3], blk[4])
                    else:
                        l3_chunk(blk[1], last=blk[1] == NB - 1)
                else:
                    dummy_mms(blk[1], BN)
    nc.compile()
    return nc


def _quant_pair(x32):
    """fp8 value+residual pair for an already-scaled f32 array."""
    hi = np.clip(x32, -240.0, 240.0).astype(FP8_NP)
    lo = (x32 - hi.astype(np.float32)).astype(FP8_NP)
    return hi, lo


def kernel(_trace=False, **inputs):
    global _cached_nc, LAST_RESULTS
    x = np.ascontiguousarray(inputs["x"], dtype=np.float32)
    if _cached_nc is None:
        _cached_nc = _build()
    nc = _cached_nc

    out_sizes = [HID, HID, HID, OUT]
    in_sizes = [IN, HID, HID, HID]
    # op1 biases ride at the relevant PSUM scale
    bias_scale = [SIGMA[0] * TAU[0], RIDE, RIDE, 1.0]
    xs = np.ascontiguousarray(x.T) * np.float32(TAU[0])
    xh, xe = _quant_pair(xs)
    xa = np.ascontiguousarray(xh[:, :BN])
    xb = np.concatenate([xh[:, BN:], xe], axis=1)
    in_maps = []
    for e in range(N_PAR):
        m = {"xa": xa, "xb": xb}
        bm = np.zeros((P, 18), dtype=np.float32)
        for l in range(4):
            r0, c0 = e * out_sizes[l], e * in_sizes[l]
            blk = np.asarray(
                inputs[f"W{l}"][r0 : r0 + out_sizes[l], c0 : c0 + in_sizes[l]]
            ).astype(np.float32)
            wt = np.ascontiguousarray(blk.T)
            if l < 2:
                a, r = _quant_pair(wt * np.float32(SIGMA[l]))
                m[f"ar{l}"] = np.concatenate([a, r], axis=1)
            else:
                m[f"w{l}"] = wt.astype(BF16_NP)
            b = np.asarray(inputs[f"b{l}"][r0 : r0 + out_sizes[l]]).astype(
                np.float32
            )
            braw = b.reshape(out_sizes[l] // P, P).T
            bm[:, BIAS_OFF[l] : BIAS_OFF[l] + braw.shape[1]] = (
                braw * np.float32(bias_scale[l])
            )
            if l == 0:
                # bias at tau1 scale for the direct-H ScalarE op
                bm[:, 14:18] = braw * np.float32(TAU[1])
        m["bm"] = bm
        in_maps.append(m)

    try:
        res = run_bass_kernel_spmd(
            nc, in_maps, core_ids=list(range(N_PAR)), trace=_trace
        )
    except Exception:
        # transient device errors clear after the runtime re-initializes
        import time

        time.sleep(30)
        res = run_bass_kernel_spmd(
            nc, in_maps, core_ids=list(range(N_PAR)), trace=_trace
        )
    LAST_RESULTS = res
    y_p = np.concatenate(
        [res.results[e]["yt"].astype(np.float32).T for e in range(N_PAR)],
        axis=1,
    ).astype(np.float32)
    x_p = np.tile(x, (1, N_PAR)).astype(np.float32)
    return (y_p, x_p)


# revision 5
# speedup vs baseline: 1.0502x; 1.0124x over previous
"""Block-diagonal 4-layer MLP (8 experts) on 8 Trainium2 NeuronCores.

Expert-parallel. Layers 0-1 run fp8e4m3 DoubleRow matmuls (2 k-planes per
instruction, 0.5 cycles/row — 4x fp32r throughput in the TRN2 cost model);
layers 2-3 run bf16. The fp8 layers use value+residual quantization so each
product is computed to ~2^-8 accuracy as three PSUM-accumulated terms:

    W·σ ≈ A + B   (A = fp8(W·σ), B = fp8(W·σ − A))
    h·τ ≈ H + E   (H = fp8(ht·s), E = ht·s − H)
    psum = A@H + B@H + A@E   (all at scale σ·τ)

x's pair is built on the host (free). h1 needs three ops (all 2-stage, so
each is placeable on ScalarE or DVE; Pool's tensor ops don't pass the BIR
verifier): op1 ht = relu(psum + b·στ) (bf16, PSUM scale riding), op2
H = fp8(ht·s), op3 E = ht·s − H (scalar_tensor_tensor). h2/h3 are single
relu-adds kept bf16 at the riding scale σ₁τ₁ = 2^18 — bf16's exponent range
makes the scale free — and layer 3's bias-add unscales by 2^-18 and stores
y in bf16, upcast on the host.

The matmul stream is scheduled as blocks (fp8 term x batch-chunk, bf16
chunk), so a late input only delays its own term; A/B and xh/xe are packed
into shared DRAM tensors to minimize serialized HWDGE descriptor-generation
slots at startup. A tiny memset feeds dummy matmuls that ramp the PE
p-state under the first DMAs; the last two output stores issue from
different queues so their SEQ-side costs overlap.
"""

import sys

import numpy as np

for _p in ("/opt/trn_rl_repo", "/root/.axon_site/_ro/trn_rl_repo"):
    if _p not in sys.path:
        sys.path.append(_p)

import ml_dtypes  # noqa: E402

import concourse.bass as bass  # noqa: E402, F401
import concourse.tile as tile  # noqa: E402
from concourse import bacc, mybir  # noqa: E402
from concourse.bass_utils import run_bass_kernel_spmd  # noqa: E402

N_PAR = 8
IN, HID, OUT, B = 256, 512, 256, 1024
P = 128
BN = 512  # batch chunk = one fp32 PSUM bank
NB = B // BN
F32 = mybir.dt.float32
BF16 = mybir.dt.bfloat16
BF16_NP = ml_dtypes.bfloat16
FP8 = mybir.dt.float8e4
FP8_NP = ml_dtypes.float8_e4m3
DR = mybir.MatmulPerfMode.DoubleRow
ADD = mybir.AluOpType.add
MAX = mybir.AluOpType.max
MULT = mybir.AluOpType.mult
SUB = mybir.AluOpType.subtract
# (K, M) of each layer's W^T
DIMS = [(IN, HID), (HID, HID), (HID, HID), (HID, OUT)]
BIAS_OFF = [0, 4, 8, 12]  # column offsets in the merged bias tile [P, 14]

# power-of-2 scales; ranges sized for W ~ U(+-1/sqrt(in_f)), x ~ N(0,1)
SIGMA = [2.0**12, 2.0**12]  # fp8 weight scale, layers 0-1
TAU = [2.0**4, 2.0**6, 2.0**9]  # fp8 act scale: x, h1, h3
S0 = TAU[1] / (SIGMA[0] * TAU[0])  # h1 rescale in op2/op3
RIDE = SIGMA[1] * TAU[1]  # h2/psum2 ride at this scale (2^18)
S3 = TAU[2] / RIDE  # h3 fp8 rescale
SIGMA3 = 2.0**12
YS = SIGMA3 * TAU[2]  # psum3 scale (2^21)

_cached_nc = None
LAST_RESULTS = None

# DMA plan: (kind, lo, hi, queue). kind xq: packed [xh | xe] col range;
# ar{l}: packed [A | R] col range; w2/w3/bm whole. queue 0=SP, 1=ACT.
DEFAULT_DMA_PLAN = [
    ("ar0", 0, 512, 1), ("xa", 0, 0, 0), ("ar0", 512, 1024, 1),
    ("xb", 0, 1536, 0),
    ("ar1", 0, 512, 0), ("bm", 0, 0, 0), ("ar1", 512, 1024, 0),
    ("w2", 0, 0, 0), ("ar3", 0, 0, 0),
]

# blocks: ("mm", l, term, n) fp8 term-block (l 0-1); ("acts1", n) h1 ops;
# ("acts2", n) h2 op; ("mmb", n) bf16 L2 chunk; ("acts3", n) h3 op;
# ("l3", n); ("fill", count)
DEFAULT_BLOCKS = [
    ("mm", 0, 0, 0), ("fill", 1), ("mm", 0, 1, 0), ("fill", 1),
    ("mm", 0, 2, 0), ("acts1a", 0),
    ("mm", 0, 0, 1), ("mm", 0, 1, 1), ("mm", 0, 2, 1), ("acts1a", 1),
    ("acts1b", 0), ("mm", 1, 0, 0), ("mm", 1, 1, 0), ("acts1b", 1),
    ("mm", 1, 2, 0), ("acts2", 0),
    ("mm", 1, 0, 1), ("mm", 1, 1, 1), ("mm", 1, 2, 1), ("acts2", 1),
    ("mmb", 0), ("acts3", 0),
    ("mmb", 1), ("l3", 0), ("acts3", 1),
    ("l3", 1, 0, 256, False), ("l3", 1, 256, 512, True),
]

# engine assignment (a=ScalarE, v=DVE only — Pool can't run these ops).
# op2 'A' = direct H from PSUM on ScalarE: relu(psum·s + b·τ) in one op
# (relu commutes with the positive scale), skipping the ht dependency.
OP1 = "avav avav".replace(" ", "")  # h1 op1 (ht), tile = 4*n + m
OP2 = "vaAa aAva".replace(" ", "")  # h1 op2 (H)
H2E = "avaa avav".replace(" ", "")  # h2 tiles
H3E = "avav avav".replace(" ", "")  # h3 tiles
FINE = "aavvav"  # L3 bias-adds: chunk0, piece1, piece2
WARM = (10, 64)  # (dummy matmul count, narrow width for the first 5)


def _build(warm=WARM, dma_plan=None, blocks=None, op1=OP1, op2=OP2,
           h2e=H2E, h3e=H3E, fine=FINE, psum_bufs=7, kouter=False):
    nc = bacc.Bacc(
        trn_type="TRN2",
        target_bir_lowering=False,
        debug=False,
        num_devices=N_PAR,
    )
    ar_aps = [
        nc.dram_tensor(f"ar{l}", [k, 2 * m], FP8, kind="ExternalInput").ap()
        for l, (k, m) in enumerate(DIMS[:2])
    ]
    w2_ap = nc.dram_tensor("w2", [DIMS[2][0], DIMS[2][1]], BF16,
                           kind="ExternalInput").ap()
    ar3_ap = nc.dram_tensor("ar3", [DIMS[3][0], 2 * DIMS[3][1]], FP8,
                            kind="ExternalInput").ap()
    xa_ap = nc.dram_tensor("xa", [IN, BN], FP8, kind="ExternalInput").ap()
    xb_ap = nc.dram_tensor("xb", [IN, 2 * B - BN], FP8,
                           kind="ExternalInput").ap()
    bm_ap = nc.dram_tensor("bm", [P, 26], F32, kind="ExternalInput").ap()
    yt = nc.dram_tensor("yt", [OUT, B], BF16, kind="ExternalOutput").ap()
    yt_t = yt.rearrange("(mt p) b -> p mt b", p=P)

    with tile.TileContext(nc) as tc:
        with (
            tc.tile_pool(name="w", bufs=1) as wpool,
            tc.tile_pool(name="acts", bufs=1) as apool,
            tc.tile_pool(name="outs", bufs=4) as opool,
            tc.tile_pool(name="psum", bufs=psum_bufs, space="PSUM") as psum,
            tc.tile_pool(name="warm", bufs=1, space="PSUM") as warmpool,
        ):
            # --- PE warmup (p-state ramp) + ACT table preload ---
            wn, ww = warm
            ww = max(ww, P)
            warm_src = apool.tile([P, BN], BF16, tag="warmsrc")
            nc.vector.memset(warm_src[:, :ww], 0.0)
            if ww < BN:
                nc.vector.memset(warm_src[:, ww:], 0.0)
            warm_ps = warmpool.tile([P, BN], F32, tag="warmps")

            def dummy_mms(count, width=None):
                for _ in range(count):
                    nc.tensor.matmul(
                        warm_ps[:, : (width or ww)], warm_src[:, :P],
                        warm_src[:, : (width or ww)],
                        start=True, stop=True,
                    )

            # narrow dummies start the PE busy-clock ASAP and bridge the
            # second memset's latency; the rest run full-width to accumulate
            # ramp time under the input DMAs
            nn = min(wn, 5)
            dummy_mms(nn)
            if wn > nn:
                dummy_mms(wn - nn, BN)
            warm_act = apool.tile([P, 1], F32, tag="warmact")
            nc.scalar.activation(
                warm_act[:], warm_src[:, :1],
                mybir.ActivationFunctionType.Relu,
            )

            # --- SBUF allocations ---
            ar_sb = [
                wpool.tile([P, k // P, 2 * m], FP8, tag=f"ar{l}", name=f"ar{l}")
                for l, (k, m) in enumerate(DIMS[:2])
            ]
            w2_sb = wpool.tile([P, DIMS[2][0] // P, DIMS[2][1]], BF16, tag="w2")
            ar3_sb = wpool.tile([P, DIMS[3][0] // P, 2 * DIMS[3][1]], FP8,
                                tag="ar3")
            xa_sb = apool.tile([P, IN // P, BN], FP8, tag="xa")
            xb_sb = apool.tile([P, IN // P, 2 * B - BN], FP8, tag="xb")
            bm_sb = wpool.tile([P, 26], F32, tag="bm")
            ht_sb = apool.tile([P, HID // P, B], BF16, tag="ht")
            hh_sb = apool.tile([P, HID // P, B], FP8, tag="hh")
            he_sb = apool.tile([P, HID // P, B], FP8, tag="he")
            h2_sb = apool.tile([P, HID // P, B], BF16, tag="h2")
            h3_sb = apool.tile([P, HID // P, B], FP8, tag="h3")
            ht3_sb = apool.tile([P, HID // P, B], BF16, tag="ht3")

            ar_t = [ar_aps[l].rearrange("(kt p) m -> p kt m", p=P)
                    for l in range(2)]
            w2_t = w2_ap.rearrange("(kt p) m -> p kt m", p=P)
            ar3_t = ar3_ap.rearrange("(kt p) m -> p kt m", p=P)
            xa_t = xa_ap.rearrange("(kt p) b -> p kt b", p=P)
            xb_t = xb_ap.rearrange("(kt p) b -> p kt b", p=P)
            _q = [nc.sync, nc.scalar]

            for kind, lo, hi, q in dma_plan or DEFAULT_DMA_PLAN:
                eng = _q[q]
                if kind == "bm":
                    eng.dma_start(bm_sb[:], bm_ap)
                elif kind == "w2":
                    eng.dma_start(w2_sb[:], w2_t)
                elif kind == "ar3":
                    eng.dma_start(ar3_sb[:], ar3_t)
                elif kind == "xa":
                    eng.dma_start(xa_sb[:], xa_t)
                elif kind == "xb":
                    eng.dma_start(xb_sb[:, :, lo:hi], xb_t[:, :, lo:hi])
                else:
                    l = int(kind[2])
                    eng.dma_start(ar_sb[l][:, :, lo:hi], ar_t[l][:, :, lo:hi])

            relu = mybir.ActivationFunctionType.Relu
            engs = {"v": nc.vector}
            pstate = {}

            def mm_block(l, term, n):
                """One fp8 term-block: 4 m-groups x kp DoubleRow matmuls."""
                kt = DIMS[l][0] // P
                kp = kt // 2
                m_w = DIMS[l][1]
                bsl = slice(n * BN, (n + 1) * BN)
                nmm = 3 * kp
                for m in range(4):
                    key = (l, n, m)
                    if key not in pstate:
                        pstate[key] = [
                            psum.tile([P, BN], F32, tag="ps", name="ps"), 0
                        ]
                    ps, cnt = pstate[key]
                    for k in range(kp):
                        if term == 0:  # A (x) H
                            wsl = slice(m * P, (m + 1) * P)
                        elif term == 1:  # B (x) H
                            wsl = slice(m_w + m * P, m_w + (m + 1) * P)
                        else:  # A (x) E
                            wsl = slice(m * P, (m + 1) * P)
                        if l == 0:
                            # xa = xh n0; xb = [xh n1 | xe n0 | xe n1]
                            if term != 2 and n == 0:
                                xs = xa_sb[:, 2 * k : 2 * k + 2, :]
                            else:
                                xoff = 0 if term != 2 else BN * (1 + n)
                                xs = xb_sb[:, 2 * k : 2 * k + 2,
                                           xoff : xoff + BN]
                        else:
                            xsb = he_sb if term == 2 else hh_sb
                            xs = xsb[:, 2 * k : 2 * k + 2, bsl]
                        nc.tensor.matmul(
                            ps[:],
                            ar_sb[l][:, 2 * k : 2 * k + 2, wsl],
                            xs,
                            start=(cnt == 0),
                            stop=(cnt == nmm - 1),
                            perf_mode=DR,
                        )
                        cnt += 1
                    pstate[key][1] = cnt

            def op1_issue(eng, dst, ps, bias):
                if eng == "a":
                    nc.scalar.activation(dst, ps[:], relu, bias=bias)
                else:
                    engs[eng].tensor_scalar(dst, ps[:], bias, 0.0, ADD, MAX)

            def acts1a_block(n):
                # H wave first (the critical dependency), then ht wave.
                # op2 'A' reads PSUM directly: H = relu(psum*s + b*tau).
                bsl = slice(n * BN, (n + 1) * BN)
                s = float(S0)
                for m in range(4):
                    t = 4 * n + m
                    ps = pstate[(0, n, m)][0]
                    hh_o = hh_sb[:, m, bsl]
                    if op2[t] == "A":
                        nc.scalar.activation(
                            hh_o, ps[:], relu,
                            bias=bm_sb[:, 14 + m : 15 + m], scale=s,
                        )
                for m in range(4):
                    t = 4 * n + m
                    ps = pstate[(0, n, m)][0]
                    bias = bm_sb[:, m : m + 1]
                    op1_issue(op1[t], ht_sb[:, m, bsl], ps, bias)
                for m in range(4):
                    t = 4 * n + m
                    pstate.pop((0, n, m))
                    ht = ht_sb[:, m, bsl]
                    hh_o = hh_sb[:, m, bsl]
                    if op2[t] == "A":
                        pass
                    elif op2[t] == "a":
                        nc.scalar.activation(
                            hh_o, ht, mybir.ActivationFunctionType.Copy,
                            scale=s,
                        )
                    else:
                        engs[op2[t]].tensor_scalar(hh_o, ht, s, None, MULT)

            def acts1b_block(n):
                # op3 wave (E — only needed by the A@E term, issued late);
                # paired m-groups per op to amortize fixed overheads
                bsl = slice(n * BN, (n + 1) * BN)
                s = float(S0)
                for m in (0, 2):
                    nc.vector.scalar_tensor_tensor(
                        he_sb[:, m : m + 2, bsl], ht_sb[:, m : m + 2, bsl], s,
                        hh_sb[:, m : m + 2, bsl], MULT, SUB,
                    )

            def acts2_block(n):
                bsl = slice(n * BN, (n + 1) * BN)
                for m in range(4):
                    ps = pstate.pop((1, n, m))[0]
                    bias = bm_sb[:, 4 + m : 4 + m + 1]
                    op1_issue(h2e[4 * n + m], h2_sb[:, m, bsl], ps, bias)

            def mmb_block(n):
                """bf16 L2 chunk: 4 m-groups x 4 k matmuls."""
                kt = DIMS[2][0] // P
                bsl = slice(n * BN, (n + 1) * BN)
                for m in range(4):
                    pstate[(2, n, m)] = [
                        psum.tile([P, BN], F32, tag="ps", name="ps"), 0
                    ]
                ordr = ([(k, m) for k in range(kt) for m in range(4)]
                        if kouter else
                        [(k, m) for m in range(4) for k in range(kt)])
                for k, m in ordr:
                    msl = slice(m * P, (m + 1) * P)
                    nc.tensor.matmul(
                        pstate[(2, n, m)][0][:],
                        w2_sb[:, k, msl], h2_sb[:, k, bsl],
                        start=(k == 0), stop=(k == kt - 1),
                    )

            def acts3_block(n):
                bsl = slice(n * BN, (n + 1) * BN)
                for m in range(4):
                    ps = pstate.pop((2, n, m))[0]
                    if h3e[4 * n + m] == "a":
                        # direct fp8: relu(psum*s + b2*tau3)
                        nc.scalar.activation(
                            h3_sb[:, m, bsl], ps[:], relu,
                            bias=bm_sb[:, 22 + m : 23 + m], scale=float(S3),
                        )
                    else:
                        bias = bm_sb[:, 8 + m : 8 + m + 1]
                        nc.vector.tensor_scalar(
                            ht3_sb[:, m, bsl], ps[:], bias, 0.0, ADD, MAX)
                        nc.vector.tensor_scalar(
                            h3_sb[:, m, bsl], ht3_sb[:, m, bsl], float(S3),
                            None, MULT)

            l3ix = [0]

            def l3_chunk(n, lo=0, hi=BN, last=False):
                """bf16 layer 3 on cols [n*BN+lo, n*BN+hi), store pipelined.
                The last piece's two stores go out on different queues so
                their SEQ-side issue overlaps."""
                kp = DIMS[3][0] // P // 2
                m_w = DIMS[3][1]
                w_ = hi - lo
                bsl = slice(n * BN + lo, n * BN + hi)
                pss = [psum.tile([P, w_], F32, tag="ps", name="ps")
                       for _ in range(2)]
                ordr = [(t_, k, m) for t_ in range(2) for k in range(kp)
                        for m in range(2)]
                nmm = 2 * kp
                cnts = [0, 0]
                for t_, k, m in ordr:
                    wsl = slice(t_ * m_w + m * P, t_ * m_w + (m + 1) * P)
                    nc.tensor.matmul(
                        pss[m][:],
                        ar3_sb[:, 2 * k : 2 * k + 2, wsl],
                        h3_sb[:, 2 * k : 2 * k + 2, bsl],
                        start=(cnts[m] == 0), stop=(cnts[m] == nmm - 1),
                        perf_mode=DR,
                    )
                    cnts[m] += 1
                o = opool.tile([P, 2, w_], BF16, tag="o", name="o")
                fbase = l3ix[0]
                l3ix[0] += 2
                for m in range(2):
                    bias = bm_sb[:, 12 + m : 12 + m + 1]
                    ps = pss[m]
                    e = fine[fbase + m]
                    if e == "a":
                        nc.scalar.activation(
                            o[:, m], ps[:],
                            mybir.ActivationFunctionType.Identity,
                            bias=bias, scale=float(1.0 / YS),
                        )
                    else:
                        engs[e].tensor_scalar(
                            o[:, m], ps[:], float(1.0 / YS), bias, MULT, ADD
                        )
                # one store covers both m-groups (their fins ran in parallel)
                nc.sync.dma_start(yt_t[:, :, bsl], o[:])

            for blk in blocks or DEFAULT_BLOCKS:
                if blk[0] == "mm":
                    mm_block(blk[1], blk[2], blk[3])
                elif blk[0] == "acts1a":
                    acts1a_block(blk[1])
                elif blk[0] == "acts1b":
                    acts1b_block(blk[1])
                elif blk[0] == "acts2":
                    acts2_block(blk[1])
                elif blk[0] == "mmb":
                    mmb_block(blk[1])
                elif blk[0] == "acts3":
                    acts3_block(blk[1])
                elif blk[0] == "l3":
                    if len(blk) > 2:
                        l3_chunk(blk[1], blk[2], blk[3], blk[4])
                    else:
                        l3_chunk(blk[1], last=blk[1] == NB - 1)
                else:
                    dummy_mms(blk[1], BN)
    nc.compile()
    return nc


def _quant_pair(x32):
    """fp8 value+residual pair for an already-scaled f32 array."""
    hi = np.clip(x32, -240.0, 240.0).astype(FP8_NP)
    lo = (x32 - hi.astype(np.float32)).astype(FP8_NP)
    return hi, lo


def kernel(_trace=False, **inputs):
    global _cached_nc, LAST_RESULTS
    x = np.ascontiguousarray(inputs["x"], dtype=np.float32)
    if _cached_nc is None:
        _cached_nc = _build()
    nc = _cached_nc

    out_sizes = [HID, HID, HID, OUT]
    in_sizes = [IN, HID, HID, HID]
    # op1 biases ride at the relevant PSUM scale
    bias_scale = [SIGMA[0] * TAU[0], RIDE, RIDE, 1.0]
    xs = np.ascontiguousarray(x.T) * np.float32(TAU[0])
    xh, xe = _quant_pair(xs)
    xa = np.ascontiguousarray(xh[:, :BN])
    xb = np.concatenate([xh[:, BN:], xe], axis=1)
    in_maps = []
    for e in range(N_PAR):
        m = {"xa": xa, "xb": xb}
        bm = np.zeros((P, 26), dtype=np.float32)
        for l in range(4):
            r0, c0 = e * out_sizes[l], e * in_sizes[l]
            blk = np.asarray(
                inputs[f"W{l}"][r0 : r0 + out_sizes[l], c0 : c0 + in_sizes[l]]
            ).astype(np.float32)
            wt = np.ascontiguousarray(blk.T)
            if l < 2 or l == 3:
                sc = SIGMA3 if l == 3 else SIGMA[l]
                a, r = _quant_pair(wt * np.float32(sc))
                m[f"ar{l}"] = np.concatenate([a, r], axis=1)
            else:
                m[f"w{l}"] = wt.astype(BF16_NP)
            b = np.asarray(inputs[f"b{l}"][r0 : r0 + out_sizes[l]]).astype(
                np.float32
            )
            braw = b.reshape(out_sizes[l] // P, P).T
            bm[:, BIAS_OFF[l] : BIAS_OFF[l] + braw.shape[1]] = (
                braw * np.float32(bias_scale[l])
            )
            if l == 0:
                # bias at tau1 scale for the direct-H ScalarE op
                bm[:, 14:18] = braw * np.float32(TAU[1])
            elif l == 2:
                bm[:, 22:26] = braw * np.float32(TAU[2])
        m["bm"] = bm
        in_maps.append(m)

    try:
        res = run_bass_kernel_spmd(
            nc, in_maps, core_ids=list(range(N_PAR)), trace=_trace
        )
    except Exception:
        # transient device errors clear after the runtime re-initializes
        import time

        time.sleep(30)
        res = run_bass_kernel_spmd(
            nc, in_maps, core_ids=list(range(N_PAR)), trace=_trace
        )
    LAST_RESULTS = res
    y_p = np.concatenate(
        [res.results[e]["yt"].astype(np.float32).T for e in range(N_PAR)],
        axis=1,
    ).astype(np.float32)
    x_p = np.tile(x, (1, N_PAR)).astype(np.float32)
    return (y_p, x_p)
